# revision 15
# baseline (speedup 1.0000x reference)
"""Causal self-attention (GPT-style, B=4 T=2048 C=768 H=12) on 8 trn2 cores.

Sharding: core = (batch b, head-group g) with g in {0,1} covering 6 heads.
Each core computes qkv projections for its 6 heads, causal flash-style
attention, and a partial c_proj (its 384 contraction rows).  The pair of
cores holding the same batch produce partial sums; the host adds them
(tensor-parallel unshard) and adds b_proj.

Device dataflow (per core), everything fp32 stored / float32r matmuls:
  x^T [768,2048] (host-pretransposed) -> Q^T,K^T [d-major], V [token-major]
  S^T[k,q] = K Q^T per head (two heads packed in the 128-row PE array)
  P^T = exp(S^T/8) on ScalarE (PSUM->SBUF), causal triangle masked by DVE
  y^T[d,q] (+row sums r[q]) = [V | ones] col-tiled matmuls vs P^T
  y_norm^T = y^T * (1/r)  -> proj: out[t,e] = sum_f y^T[f,t] wp[f,e]
"""

from contextlib import ExitStack

import numpy as np

import concourse.bass as bass
import concourse.mybir as mybir
import concourse.tile as tile
from concourse import bacc
from concourse.masks import make_lower_triangular

AF = mybir.ActivationFunctionType
F32 = mybir.dt.float32
F32R = mybir.dt.float32r

C = 768          # model dim
D = 64           # head dim
HG = 6           # heads per core
NP = 3           # head pairs per core
GC = HG * D      # 384 group channels
CT = C // 128    # 6 contraction tiles
QBLK = 512       # query tile (psum bank)
KBLK = 128       # key tile (partition dim)




def build_nc(T=2048):
    NQ = T // QBLK
    NK = T // KBLK
    nc = bacc.Bacc(None)

    xt_d = nc.dram_tensor("xt", [C, T], F32R, kind="ExternalInput")
    wa_d = nc.dram_tensor("wa", [C, 3 * GC], F32R, kind="ExternalInput")
    bqk_d = nc.dram_tensor("bqk", [128, 2, NP], F32, kind="ExternalInput")
    wp_d = nc.dram_tensor("wp", [GC, C], F32R, kind="ExternalInput")
    out_d = nc.dram_tensor("out", [T, C], F32, kind="ExternalOutput")

    with ExitStack() as ctx:
        tc = ctx.enter_context(tile.TileContext(nc))
        const = ctx.enter_context(tc.tile_pool(name="const", bufs=1))
        big = ctx.enter_context(tc.tile_pool(name="big", bufs=1))
        qtp = ctx.enter_context(tc.tile_pool(name="qtp", bufs=2))
        ytp = ctx.enter_context(tc.tile_pool(name="ytp", bufs=2))
        ptp = ctx.enter_context(tc.tile_pool(name="ptp", bufs=2))
        rp = ctx.enter_context(tc.tile_pool(name="rp", bufs=1))
        obp = ctx.enter_context(tc.tile_pool(name="obp", bufs=1))
        psA = ctx.enter_context(tc.tile_pool(name="psA", bufs=1, space="PSUM"))
        psY = ctx.enter_context(tc.tile_pool(name="psY", bufs=2, space="PSUM"))
        psQ = ctx.enter_context(tc.tile_pool(name="psQ", bufs=2, space="PSUM"))

        ones_f = const.tile([128, NP, D], F32)
        nc.vector.memset(ones_f, 1.0)
        # additive causal mask: -1e5 strictly below the diagonal
        mask_sb = const.tile([128, KBLK], F32)
        make_lower_triangular(nc, mask_sb, val=-1e5, diag=False)
        bqk_sb = const.tile([128, 2, NP], F32)
        nc.gpsimd.dma_start(out=bqk_sb, in_=bqk_d[:, :, :])

        xt = big.tile([128, CT, T], F32R)
        wa = big.tile([128, CT, 3 * GC], F32R)
        wp = big.tile([128, NP, C], F32R)
        kt = big.tile([128, NP, T], F32R)
        # V interleaved with ones columns: even head h -> [V_h | 1],
        # odd head h -> [1 | V_h]; a single M=128 matmul then yields
        # y^T on one 64-partition half and the exp row-sums on the other.
        vs = big.tile([128, NK, HG, 2 * D], F32R)

        xt_r = xt_d[:, :].rearrange("(ct r) t -> ct r t", r=128)
        wa_r = wa_d[:, :].rearrange("(ct r) j -> ct r j", r=128)
        for ct in range(CT):
            nc.gpsimd.dma_start(out=xt[:, ct, :], in_=xt_r[ct])
            nc.gpsimd.dma_start(out=wa[:, ct, :], in_=wa_r[ct])
        wp_r = wp_d[:, :].rearrange("(p r) e -> p r e", r=128)
        for p in range(NP):
            nc.gpsimd.dma_start(out=wp[:, p, :], in_=wp_r[p])

        # V = x @ Wv, token-major [k, d], interleaved with ones blocks
        for k_i in range(NK):
            pv = psQ.tile([128, GC], F32, tag="pq", name="pv")
            for ct in range(CT):
                nc.tensor.matmul(
                    pv,
                    lhsT=xt[:, ct, k_i * KBLK:(k_i + 1) * KBLK],
                    rhs=wa[:, ct, 2 * GC:3 * GC],
                    start=(ct == 0), stop=(ct == CT - 1))
            pv3 = pv.rearrange("r (a b d) -> r a b d", b=2, d=D)
            vs4 = vs[:, k_i].rearrange("r (a b) e -> r a b e", b=2)
            nc.scalar.copy(vs4[:, :, 0, 0:D], pv3[:, :, 0, :])
            nc.scalar.copy(vs4[:, :, 1, D:2 * D], pv3[:, :, 1, :])
            nc.vector.tensor_copy(vs4[:, :, 0, D:2 * D], ones_f)
            nc.vector.tensor_copy(vs4[:, :, 1, 0:D], ones_f)

        for q in range(NQ):
            qs = q * QBLK
            qt = qtp.tile([128, NP, QBLK], F32R, tag="qt", name="qt")
            yt = ytp.tile([128, NP, QBLK], F32R, tag="yt", name="yt")
            # Q^T / K^T (d-major) for this q-range, all pairs
            for p in range(NP):
                for which in (0, 1):
                    pqk = psQ.tile([128, QBLK], F32, tag="pq", name="pqk")
                    for ct in range(CT):
                        nc.tensor.matmul(
                            pqk,
                            lhsT=wa[:, ct, which * GC + p * 128:
                                           which * GC + (p + 1) * 128],
                            rhs=xt[:, ct, qs:qs + QBLK],
                            start=(ct == 0), stop=(ct == CT - 1))
                    if which == 0:
                        nc.scalar.add(qt[:, p, :], pqk, bqk_sb[:, 0, p:p + 1])
                    else:
                        nc.scalar.add(kt[:, p, qs:qs + QBLK], pqk,
                                      bqk_sb[:, 1, p:p + 1])

            for p in range(NP):
                nkt = (q + 1) * (QBLK // KBLK)
                ya = psY.tile([128, QBLK], F32, tag="y", name="ya")
                yb = psY.tile([128, QBLK], F32, tag="y", name="yb")
                for kg in range(0, nkt, 2):
                    ks = (kg, kg + 1)
                    st = psA.tile([128, 4 * QBLK], F32, tag="st", name="st")
                    pt = ptp.tile([128, 4 * QBLK], F32R, tag="pt", name="pt")
                    # S^T = K·Q^T, both heads row-packed (contraction d=64)
                    for j, k_i in enumerate(ks):
                        for s in range(2):
                            hoff = 64 * s
                            slot = 2 * j + s
                            nc.tensor.matmul(
                                st[:, slot * QBLK:(slot + 1) * QBLK],
                                lhsT=kt[hoff:hoff + 64, p,
                                           k_i * KBLK:(k_i + 1) * KBLK],
                                rhs=qt[hoff:hoff + 64, p, :],
                                start=True, stop=True)
                    # additive causal mask on diagonal k-tiles (pre-exp)
                    for j, k_i in enumerate(ks):
                        m = k_i - 4 * q
                        if m >= 0:
                            col0 = m * KBLK
                            for s in range(2):
                                slot = 2 * j + s
                                seg = st[:, slot * QBLK + col0:
                                            slot * QBLK + col0 + KBLK]
                                nc.vector.tensor_add(seg, seg, mask_sb)
                    nc.scalar.activation(pt, st, AF.Exp, scale=0.125)
                    # y^T + row sums in one M=128 matmul per (head, ktile)
                    for j, k_i in enumerate(ks):
                        m = k_i - 4 * q
                        col0 = max(m, 0) * KBLK
                        first = (k_i == 0)
                        last = (k_i == nkt - 1)
                        for s in range(2):
                            slot = 2 * j + s
                            h = 2 * p + s
                            rhs = pt[:, slot * QBLK + col0:
                                           (slot + 1) * QBLK]
                            yy = ya if s == 0 else yb
                            nc.tensor.matmul(
                                yy[:, col0:QBLK],
                                lhsT=vs[:, k_i, h, :], rhs=rhs,
                                start=first, stop=last,
                                skip_group_check=True)
                # normalize: y^T / r  (r halves partition-shifted via DMA)
                rtmp = rp.tile([128, QBLK], F32, tag="rtmp", name="rtmp")
                nc.vector.tensor_copy(rtmp[64:128, :], ya[64:128, :])
                nc.vector.tensor_copy(rtmp[0:64, :], yb[0:64, :])
                rsh = rp.tile([128, QBLK], F32, tag="rsh", name="rsh")
                nc.sync.dma_start(out=rsh[0:64, :], in_=rtmp[64:128, :])
                nc.sync.dma_start(out=rsh[64:128, :], in_=rtmp[0:64, :])
                rec = rp.tile([128, QBLK], F32, tag="rec", name="rec")
                scr = rp.tile([128, QBLK], F32, tag="rtmp", name="scr")
                nc.vector.reciprocal_approx_accurate(rec, rsh, scr)
                nc.vector.tensor_mul(yt[0:64, p, :], ya[0:64, :],
                                     rec[0:64, :])
                nc.vector.tensor_mul(yt[64:128, p, :], yb[64:128, :],
                                     rec[64:128, :])

            # partial c_proj for this q-range
            for tt in range(QBLK // KBLK):
                t0 = qs + tt * KBLK
                ob = obp.tile([128, C], F32, tag="ob", name="ob")
                for ec in range(2):
                    po = psQ.tile([128, GC], F32, tag="pq", name="po")
                    for j in range(NP):
                        nc.tensor.matmul(
                            po,
                            lhsT=yt[:, j, tt * KBLK:(tt + 1) * KBLK],
                            rhs=wp[:, j, ec * GC:(ec + 1) * GC],
                            start=(j == 0), stop=(j == NP - 1))
                    nc.vector.tensor_copy(ob[:, ec * GC:(ec + 1) * GC], po)
                nc.sync.dma_start(out=out_d[t0:t0 + KBLK, :], in_=ob)
    nc.compile()
    return nc


def make_in_map(x_b, w_attn, b_attn, w_proj, g):
    """Per-core input arrays for batch slice x_b and head-group g."""
    sl = slice(g * GC, (g + 1) * GC)
    wq = w_attn[:, 0 * C:1 * C][:, sl]
    wk = w_attn[:, 1 * C:2 * C][:, sl]
    wv = w_attn[:, 2 * C:3 * C][:, sl]
    bq = b_attn[0 * C:1 * C][sl]
    bk = b_attn[1 * C:2 * C][sl]
    bv = b_attn[2 * C:3 * C][sl]
    bqk = np.ascontiguousarray(
        np.stack([bq, bk]).reshape(2, NP, 128).transpose(2, 0, 1))
    return {
        "xt": np.ascontiguousarray(x_b.T),
        "wa": np.ascontiguousarray(np.concatenate([wq, wk, wv], axis=1)),
        "bqk": bqk,
        "wp": np.ascontiguousarray(w_proj[sl, :]),
    }


_NC_CACHE = {}


def _get_nc(T):
    if T not in _NC_CACHE:
        _NC_CACHE[T] = build_nc(T)
    return _NC_CACHE[T]


def kernel(x, w_attn, b_attn, w_proj, b_proj, _trace=False):
    from concourse.bass_utils import run_bass_kernel_spmd

    x = np.asarray(x, dtype=np.float32)
    w_attn = np.asarray(w_attn, dtype=np.float32)
    b_attn = np.asarray(b_attn, dtype=np.float32)
    w_proj = np.asarray(w_proj, dtype=np.float32)
    b_proj = np.asarray(b_proj, dtype=np.float32)
    B, T, _ = x.shape

    nc = _get_nc(T)
    in_maps = []
    for b in range(B):
        for g in range(2):
            in_maps.append(make_in_map(x[b], w_attn, b_attn, w_proj, g))
    res = run_bass_kernel_spmd(nc, in_maps, core_ids=list(range(2 * B)),
                               trace=_trace)
    outs = [r["out"] for r in res.results]
    # softmax rows sum to 1, so the V-bias contribution is exactly
    # bv @ w_proj added to every token (not computed on device).
    bias_row = b_proj + b_attn[2 * C:3 * C] @ w_proj
    out = np.empty((B, T, C), dtype=np.float32)
    for b in range(B):
        out[b] = outs[2 * b] + outs[2 * b + 1] + bias_row[None, :]
    if _trace:
        kernel.last_result = res
    return out



# revision 20
# speedup vs baseline: 1.9949x; 1.9949x over previous
"""Causal self-attention (GPT-style, B=4 T=2048 C=768 H=12) on 8 trn2 cores.

Sharding: core = (batch b, head-group g) with g in {0,1} covering 6 heads.
Each core computes qkv projections for its 6 heads, causal flash-style
attention, and a partial c_proj (its 384 contraction rows).  The pair of
cores holding the same batch produce partial sums; the host adds them
(tensor-parallel unshard) and adds b_proj.

Device dataflow (per core), everything fp32 stored / float32r matmuls:
  x^T [768,2048] (host-pretransposed) -> Q^T,K^T [d-major], V [token-major]
  S^T[k,q] = K Q^T per head (two heads packed in the 128-row PE array)
  P^T = exp(S^T/8) on ScalarE (PSUM->SBUF), causal triangle masked by DVE
  y^T[d,q] (+row sums r[q]) = [V | ones] col-tiled matmuls vs P^T
  y_norm^T = y^T * (1/r)  -> proj: out[t,e] = sum_f y^T[f,t] wp[f,e]
"""

from contextlib import ExitStack

import numpy as np

import concourse.bass as bass
import concourse.mybir as mybir
import concourse.tile as tile
from concourse import bacc
from concourse.masks import make_upper_triangular

AF = mybir.ActivationFunctionType
F32 = mybir.dt.float32
F32R = mybir.dt.float32r

C = 768          # model dim
D = 64           # head dim
HG = 6           # heads per core
NP = 3           # head pairs per core
GC = HG * D      # 384 group channels
CT = C // 128    # 6 contraction tiles
QBLK = 512       # query tile (psum bank)
KBLK = 128       # key tile (partition dim)




def build_nc(T=2048):
    NQ = T // QBLK
    NK = T // KBLK
    nc = bacc.Bacc(None)

    xt_d = nc.dram_tensor("xt", [C, T], F32R, kind="ExternalInput")
    wa_d = nc.dram_tensor("wa", [C, 3 * GC], F32R, kind="ExternalInput")
    bqk_d = nc.dram_tensor("bqk", [128, 2, NP], F32, kind="ExternalInput")
    wp_d = nc.dram_tensor("wp", [GC, C], F32R, kind="ExternalInput")
    out_d = nc.dram_tensor("out", [T, C], F32, kind="ExternalOutput")

    with ExitStack() as ctx:
        tc = ctx.enter_context(tile.TileContext(nc))
        const = ctx.enter_context(tc.tile_pool(name="const", bufs=1))
        big = ctx.enter_context(tc.tile_pool(name="big", bufs=1))
        qtp = ctx.enter_context(tc.tile_pool(name="qtp", bufs=2))
        ytp = ctx.enter_context(tc.tile_pool(name="ytp", bufs=2))
        ptp = ctx.enter_context(tc.tile_pool(name="ptp", bufs=3))
        rp = ctx.enter_context(tc.tile_pool(name="rp", bufs=1))
        obp = ctx.enter_context(tc.tile_pool(name="obp", bufs=1))
        psA = ctx.enter_context(tc.tile_pool(name="psA", bufs=2, space="PSUM"))
        psY = ctx.enter_context(tc.tile_pool(name="psY", bufs=2, space="PSUM"))
        psQ = ctx.enter_context(tc.tile_pool(name="psQ", bufs=1, space="PSUM"))

        ones_f = const.tile([128, NP, D], F32)
        nc.vector.memset(ones_f, 1.0)
        # multiplicative causal mask: 1 on/above the diagonal, 0 below
        mask_sb = const.tile([128, KBLK], F32)
        make_upper_triangular(nc, mask_sb, val=1.0, diag=True)
        bqk_sb = const.tile([128, 2, NP], F32)
        nc.gpsimd.dma_start(out=bqk_sb, in_=bqk_d[:, :, :])

        xt = big.tile([128, CT, T], F32R)
        wa = big.tile([128, CT, 3 * GC], F32R)
        wp = big.tile([128, NP, C], F32R)
        kt = big.tile([128, NP, T], F32R)
        # V interleaved with ones columns: even head h -> [V_h | 1],
        # odd head h -> [1 | V_h]; a single M=128 matmul then yields
        # y^T on one 64-partition half and the exp row-sums on the other.
        vs = big.tile([128, NK, HG, 2 * D], F32R)

        xt_r = xt_d[:, :].rearrange("(ct r) t -> ct r t", r=128)
        wa_r = wa_d[:, :].rearrange("(ct r) j -> ct r j", r=128)
        for ct in range(CT):
            nc.sync.dma_start(out=xt[:, ct, :], in_=xt_r[ct])
            nc.sync.dma_start(out=wa[:, ct, :], in_=wa_r[ct])
        wp_r = wp_d[:, :].rearrange("(p r) e -> p r e", r=128)
        for p in range(NP):
            nc.sync.dma_start(out=wp[:, p, :], in_=wp_r[p])

        for q in range(NQ):
            qs = q * QBLK
            qt = qtp.tile([128, NP, QBLK], F32R, tag="qt", name="qt")
            yt = ytp.tile([128, NP, QBLK], F32R, tag="yt", name="yt")
            # Q^T / K^T (d-major) for this q-range, all pairs
            for p in range(NP):
                for which in (0, 1):
                    pqk = psQ.tile([128, QBLK], F32, tag="pq", name="pqk")
                    for ct in range(CT):
                        nc.tensor.matmul(
                            pqk,
                            lhsT=wa[:, ct, which * GC + p * 128:
                                           which * GC + (p + 1) * 128],
                            rhs=xt[:, ct, qs:qs + QBLK],
                            start=(ct == 0), stop=(ct == CT - 1))
                    if which == 0:
                        nc.vector.tensor_scalar_add(qt[:, p, :], pqk,
                                                    bqk_sb[:, 0, p:p + 1])
                    else:
                        nc.vector.tensor_scalar_add(kt[:, p, qs:qs + QBLK],
                                                    pqk,
                                                    bqk_sb[:, 1, p:p + 1])

            # V (+ interleaved ones) for this qtile's k-range
            for k_i in range(4 * q, 4 * (q + 1)):
                pv = psQ.tile([128, GC], F32, tag="pq", name="pv")
                for ct in range(CT):
                    nc.tensor.matmul(
                        pv,
                        lhsT=xt[:, ct, k_i * KBLK:(k_i + 1) * KBLK],
                        rhs=wa[:, ct, 2 * GC:3 * GC],
                        start=(ct == 0), stop=(ct == CT - 1))
                pv3 = pv.rearrange("r (a b d) -> r a b d", b=2, d=D)
                vs4 = vs[:, k_i].rearrange("r (a b) e -> r a b e", b=2)
                nc.vector.tensor_copy(vs4[:, :, 0, 0:D], pv3[:, :, 0, :])
                nc.vector.tensor_copy(vs4[:, :, 1, D:2 * D], pv3[:, :, 1, :])
                nc.vector.tensor_copy(vs4[:, :, 0, D:2 * D], ones_f)
                nc.vector.tensor_copy(vs4[:, :, 1, 0:D], ones_f)

            for p in range(NP):
                nkt = (q + 1) * (QBLK // KBLK)
                ya = psY.tile([128, QBLK], F32, tag="y", name="ya")
                yb = psY.tile([128, QBLK], F32, tag="y", name="yb")
                for k_i in range(nkt):
                    st = psA.tile([128, 2, QBLK], F32, tag="st", name="st")
                    pt = ptp.tile([128, 2, QBLK], F32R, tag="pt", name="pt")
                    m = k_i - 4 * q
                    col0 = max(m, 0) * KBLK
                    first = (k_i == 0)
                    last = (k_i == nkt - 1)
                    # S^T = K·Q^T, both heads row-packed (contraction d=64)
                    for s in range(2):
                        hoff = 64 * s
                        nc.tensor.matmul(
                            st[:, s, col0:QBLK],
                            lhsT=kt[hoff:hoff + 64, p,
                                       k_i * KBLK:(k_i + 1) * KBLK],
                            rhs=qt[hoff:hoff + 64, p, col0:QBLK],
                            start=True, stop=True)
                    nc.scalar.activation(pt[:, :, col0:QBLK],
                                         st[:, :, col0:QBLK],
                                         AF.Exp, scale=0.125)
                    # multiplicative causal mask on diagonal k-tile (gpsimd)
                    if m >= 0:
                        for s in range(2):
                            seg = pt[:, s, col0:col0 + KBLK]
                            nc.gpsimd.tensor_mul(seg, seg, mask_sb)
                    # y^T + row sums in one M=128 matmul per head
                    for s in range(2):
                        h = 2 * p + s
                        yy = ya if s == 0 else yb
                        nc.tensor.matmul(
                            yy[:, col0:QBLK],
                            lhsT=vs[:, k_i, h, :],
                            rhs=pt[:, s, col0:QBLK],
                            start=first, stop=last,
                            skip_group_check=True)
                # normalize: y^T / r.  Evict PSUM fast (frees psY slots),
                # partition-shift r via gpsimd, single-pass reciprocal.
                ya_sb = rp.tile([128, QBLK], F32, tag="ya", name="ya_sb")
                yb_sb = rp.tile([128, QBLK], F32, tag="yb", name="yb_sb")
                nc.vector.tensor_copy(ya_sb, ya)
                nc.vector.tensor_copy(yb_sb, yb)
                rsh = rp.tile([128, QBLK], F32, tag="rsh", name="rsh")
                nc.gpsimd.tensor_copy(rsh[0:64, :], ya_sb[64:128, :])
                nc.gpsimd.tensor_copy(rsh[64:128, :], yb_sb[0:64, :])
                rec = rp.tile([128, QBLK], F32, tag="rec", name="rec")
                nc.vector.reciprocal_approx_fast(rec, rsh)
                nc.vector.tensor_mul(yt[0:64, p, :], ya_sb[0:64, :],
                                     rec[0:64, :])
                nc.vector.tensor_mul(yt[64:128, p, :], yb_sb[64:128, :],
                                     rec[64:128, :])

            # partial c_proj for this q-range
            for tt in range(QBLK // KBLK):
                t0 = qs + tt * KBLK
                ob = obp.tile([128, C], F32, tag="ob", name="ob")
                for ec in range(2):
                    po = psQ.tile([128, GC], F32, tag="po", name="po")
                    for j in range(NP):
                        nc.tensor.matmul(
                            po,
                            lhsT=yt[:, j, tt * KBLK:(tt + 1) * KBLK],
                            rhs=wp[:, j, ec * GC:(ec + 1) * GC],
                            start=(j == 0), stop=(j == NP - 1))
                    nc.vector.tensor_copy(ob[:, ec * GC:(ec + 1) * GC], po)
                nc.sync.dma_start(out=out_d[t0:t0 + KBLK, :], in_=ob)
    nc.compile()
    return nc


def make_in_map(x_b, w_attn, b_attn, w_proj, g):
    """Per-core input arrays for batch slice x_b and head-group g."""
    sl = slice(g * GC, (g + 1) * GC)
    wq = w_attn[:, 0 * C:1 * C][:, sl]
    wk = w_attn[:, 1 * C:2 * C][:, sl]
    wv = w_attn[:, 2 * C:3 * C][:, sl]
    bq = b_attn[0 * C:1 * C][sl]
    bk = b_attn[1 * C:2 * C][sl]
    bv = b_attn[2 * C:3 * C][sl]
    bqk = np.ascontiguousarray(
        np.stack([bq, bk]).reshape(2, NP, 128).transpose(2, 0, 1))
    return {
        "xt": np.ascontiguousarray(x_b.T),
        "wa": np.ascontiguousarray(np.concatenate([wq, wk, wv], axis=1)),
        "bqk": bqk,
        "wp": np.ascontiguousarray(w_proj[sl, :]),
    }


_NC_CACHE = {}


def _get_nc(T):
    if T not in _NC_CACHE:
        _NC_CACHE[T] = build_nc(T)
    return _NC_CACHE[T]


def kernel(x, w_attn, b_attn, w_proj, b_proj, _trace=False):
    from concourse.bass_utils import run_bass_kernel_spmd

    x = np.asarray(x, dtype=np.float32)
    w_attn = np.asarray(w_attn, dtype=np.float32)
    b_attn = np.asarray(b_attn, dtype=np.float32)
    w_proj = np.asarray(w_proj, dtype=np.float32)
    b_proj = np.asarray(b_proj, dtype=np.float32)
    B, T, _ = x.shape

    nc = _get_nc(T)
    in_maps = []
    for b in range(B):
        for g in range(2):
            in_maps.append(make_in_map(x[b], w_attn, b_attn, w_proj, g))
    res = run_bass_kernel_spmd(nc, in_maps, core_ids=list(range(2 * B)),
                               trace=_trace)
    outs = [r["out"] for r in res.results]
    # softmax rows sum to 1, so the V-bias contribution is exactly
    # bv @ w_proj added to every token (not computed on device).
    bias_row = b_proj + b_attn[2 * C:3 * C] @ w_proj
    out = np.empty((B, T, C), dtype=np.float32)
    for b in range(B):
        out[b] = outs[2 * b] + outs[2 * b + 1] + bias_row[None, :]
    if _trace:
        kernel.last_result = res
    return out

